# revision 1
# baseline (speedup 1.0000x reference)
"""BitNet attention TRN2 kernel: 8-way SPMD (2 heads/core, tokens sharded 8-way).

Dataflow per core c (tokens Tc = rows [c*L,(c+1)*L) of the flattened [B*T, D]
activations, heads {2c, 2c+1}):
  A) x_had = x @ H  via fp16 hi/lo split matmuls (fp32 accumulate);
     per-token int8 absmax quant -> y (fp16-held small ints); PE-transpose.
     AllGathers pipelined: y-half0 (issued mid-phase), absmax vector, y-half1.
  B) Q/K/V projections in the integer domain (exact in fp16), dequantized with
     per-token scales; processed per token-half so half0 compute overlaps the
     half1 AllGather.
  C) Per (batch, head-pair, query block): S.T = Ks.T^T @ Qs.T, exp on ACT,
     out.T = [V | 1]^T @ expS.T accumulated over key tiles (ones column gives
     the softmax denominator). Unnormalized outputs + denominator rows are
     copied to SBUF and DMA'd straight into the AllToAll buffer.
  D) Per-token normalize (denominators arrived via A2A), second absmax quant,
     z = y2 @ Wo_u.T (integer domain), per-token dequant.

Host side quantizes weights to ternary {-1,0,1} (fp16-exact), splits/transposes
x, and concatenates the 8 z slices.
"""
import sys

if "/opt/trn_rl_repo" not in sys.path:
    sys.path.insert(0, "/opt/trn_rl_repo")

import numpy as np

P = 128
D = 1024
NH = 16
DH = 64
B = 2
N_CORES = 8
MAGIC = 12582912.0  # 1.5 * 2**23: fp32 round-to-nearest-int via add/sub

_BUILD_CACHE = {}


def _build(T):
    import concourse.bass as bass  # noqa: F401
    import concourse.mybir as mybir
    import concourse.tile as tile
    from concourse import bacc
    from concourse.masks import make_identity

    f16 = mybir.dt.float16
    f32 = mybir.dt.float32
    i8 = mybir.dt.int8
    Exp = mybir.ActivationFunctionType.Exp
    mult = mybir.AluOpType.mult
    add = mybir.AluOpType.add
    subtract = mybir.AluOpType.subtract
    X = mybir.AxisListType.X
    GROUPS = [list(range(N_CORES))]

    BT = B * T
    L = BT // N_CORES          # tokens per core
    NT = L // P                # local token tiles
    DK = D // P                # contraction chunks
    HL = L // 2                # tokens per gather half
    QB = 512                   # query block
    NQB = T // QB              # query blocks per batch
    NKT = T // P               # key tiles per batch
    VT = BT // P               # global token tiles (for V)
    PPQ = max(1, QB // L)      # peers spanned by one query block
    SLW = min(512, BT // 2)    # B-phase token-slice width
    NSL = (BT // 2) // SLW     # slices per half
    PPS = SLW // HL            # peers per B slice

    nc = bacc.Bacc("TRN2", target_bir_lowering=False, debug=False,
                   num_devices=N_CORES)

    # I/O
    xT_hi = nc.dram_tensor("xT_hi", [D, L], f16, kind="ExternalInput")
    xT_lo = nc.dram_tensor("xT_lo", [D, L], f16, kind="ExternalInput")
    Hm = nc.dram_tensor("Hm", [D, D], f16, kind="ExternalInput")
    WqT = nc.dram_tensor("WqT", [D, P], f16, kind="ExternalInput")
    WkT = nc.dram_tensor("WkT", [D, P], f16, kind="ExternalInput")
    WvT = nc.dram_tensor("WvT", [D, P], f16, kind="ExternalInput")
    WoT = nc.dram_tensor("WoT", [D, D], f16, kind="ExternalInput")
    consts = nc.dram_tensor("consts", [1, 4], f32, kind="ExternalInput")
    z = nc.dram_tensor("z", [L, D], f32, kind="ExternalOutput")

    with tile.TileContext(nc) as tc:
        cpool = tc.alloc_tile_pool(name="cpool", bufs=1)
        dram = tc.alloc_tile_pool(name="dram", bufs=1, space="DRAM")

        ident16 = cpool.tile([P, P], f16)
        make_identity(nc, ident16)
        ident32 = cpool.tile([P, P], f32)
        make_identity(nc, ident32)
        csb = cpool.tile([P, 4], f32)
        nc.sync.dma_start(csb, consts.ap().to_broadcast((P, 4)))

        # DRAM intermediates (y0 block carries the absmax vector as 8
        # bitcast int8 rows, so one AllGather ships both)
        AMR = 8
        yT_loc0 = dram.tile([D + AMR, HL], i8)
        yT_loc1 = dram.tile([D, HL], i8)
        yT_g0 = dram.tile([N_CORES * (D + AMR), HL], i8, addr_space="Shared")
        yT_g1 = dram.tile([N_CORES * D, HL], i8, addr_space="Shared")
        a2a_in = dram.tile([N_CORES * 130, L], f32)
        a2a_out = dram.tile([N_CORES * 130, L], f32)

        # ---------------- Phase A: x@H, quant, transpose, gather ----------
        with tc.tile_pool(name="pre", bufs=1) as pre, \
             tc.tile_pool(name="workA", bufs=3) as workA, \
             tc.tile_pool(name="psA", bufs=2, space="PSUM") as psA, \
             tc.tile_pool(name="psT", bufs=4, space="PSUM") as psT:
            sA = nc.named_scope("phaseA"); sA.__enter__()
            xhi = pre.tile([P, DK, L], f16)
            xlo = pre.tile([P, DK, L], f16)
            Hsb = pre.tile([P, DK, D], f16)
            xhi_v = xT_hi.ap().rearrange("(o p) t -> p o t", p=P)
            xlo_v = xT_lo.ap().rearrange("(o p) t -> p o t", p=P)
            H_v = Hm.ap().rearrange("(o p) d -> p o d", p=P)
            for kc in range(DK):
                nc.sync.dma_start(Hsb[:, kc], H_v[:, kc])
                nc.sync.dma_start(xhi[:, kc], xhi_v[:, kc])
                nc.sync.dma_start(xlo[:, kc], xlo_v[:, kc])
            yT_sb = pre.tile([P, DK, L], i8)
            am_all = pre.tile([P, NT], f32)

            for tt in range(NT):
                ps = psA.tile([P, 1024], f32, tag="xh")
                for half in range(2):
                    for kc in range(DK):
                        nc.tensor.matmul(
                            ps[:, half * 512:(half + 1) * 512],
                            xhi[:, kc, tt * P:(tt + 1) * P],
                            Hsb[:, kc, half * 512:(half + 1) * 512],
                            start=(kc == 0), stop=False)
                        nc.tensor.matmul(
                            ps[:, half * 512:(half + 1) * 512],
                            xlo[:, kc, tt * P:(tt + 1) * P],
                            Hsb[:, kc, half * 512:(half + 1) * 512],
                            start=False, stop=(kc == DK - 1))
                am_t = am_all[:, tt:tt + 1]
                nc.vector.reduce_max(am_t, ps, axis=X, apply_absolute_value=True)
                nc.vector.tensor_scalar_max(am_t, am_t, 1e-5)
                rec = workA.tile([P, 1], f32, tag="rec")
                nc.vector.reciprocal(rec, am_t)
                s127 = workA.tile([P, 1], f32, tag="s127")
                nc.vector.tensor_scalar_mul(s127, rec, 127.0)
                tmp = workA.tile([P, 1024], f32, tag="tmpA")
                nc.scalar.activation(tmp, ps,
                                     mybir.ActivationFunctionType.Copy,
                                     MAGIC, s127)
                y_t = workA.tile([P, 1024], f16, tag="yA")
                nc.vector.tensor_scalar(y_t, tmp, MAGIC, None, subtract)
                for kc in range(DK):
                    pst = psT.tile([P, P], f16, tag="trA")
                    nc.tensor.transpose(pst, y_t[:, kc * P:(kc + 1) * P], ident16)
                    nc.vector.tensor_copy(yT_sb[:, kc, tt * P:(tt + 1) * P], pst)
                if 2 * (tt + 1) == NT or NT == 1:
                    # first half of local tokens is quantized+transposed:
                    # stage it for the gather while the rest computes
                    nc.sync.dma_start(
                        yT_loc0[0:D, :].rearrange("(o p) t -> p o t", p=P),
                        yT_sb[:, :, 0:HL])
            sA.__exit__(None, None, None)
            sG = nc.named_scope("gather"); sG.__enter__()
            nc.sync.dma_start(
                yT_loc0[D:D + AMR, :].bitcast(f32)
                .rearrange("r j -> (r j)").rearrange("(t p) -> p t", p=P),
                am_all)
            nc.gpsimd.collective_compute(
                "AllGather", mybir.AluOpType.bypass, replica_groups=GROUPS,
                ins=[yT_loc0.opt()], outs=[yT_g0.opt()])
            nc.sync.dma_start(
                yT_loc1.rearrange("(o p) t -> p o t", p=P),
                yT_sb[:, :, HL:L])
            nc.gpsimd.collective_compute(
                "AllGather", mybir.AluOpType.bypass, replica_groups=GROUPS,
                ins=[yT_loc1.opt()], outs=[yT_g1.opt()])
            sG.__exit__(None, None, None)

        # ---------------- Phase B: QKV ------------------------------------
        fin = tc.alloc_tile_pool(name="fin", bufs=1)
        attn = tc.alloc_tile_pool(name="attn", bufs=1)
        # token-ordered flat [P, BT]; 4-D (peer, half, HL) views for B writes
        QsT = attn.tile([P, BT], f16)
        KsT = attn.tile([P, BT], f16)
        V_A = attn.tile([P, VT, 65], f16)
        V_B = attn.tile([P, VT, 65], f16)

        with tc.tile_pool(name="gath", bufs=1) as gath, \
             tc.tile_pool(name="workB", bufs=4) as workB, \
             tc.tile_pool(name="psQK", bufs=2, space="PSUM") as psQK, \
             tc.tile_pool(name="psV", bufs=2, space="PSUM") as psV:
            sB = nc.named_scope("phaseB"); sB.__enter__()
            yTg = gath.tile([P, DK, N_CORES, 2, HL], f16)
            yg0 = yT_g0.rearrange("(a r) t -> a r t", r=D + AMR)
            yv1 = yT_g1.rearrange("(a o p) t -> a p o t", p=P, o=DK)

            A_q = gath.tile([P, BT], f32)
            A_k = gath.tile([P, BT], f32)
            Av = gath.tile([P, VT], f32)
            for peer in range(N_CORES):
                amv = (yg0[peer, D:D + AMR, :].bitcast(f32)
                       .rearrange("r j -> (r j)"))
                nc.sync.dma_start(
                    A_k[:, peer * L:(peer + 1) * L],
                    amv[None, :].to_broadcast((P, L)))
                nc.sync.dma_start(
                    Av[:, peer * NT:(peer + 1) * NT],
                    amv.rearrange("(t p) -> p t", p=P))
            Q4 = QsT[:, :].rearrange("p (a h t) -> p a h t", a=N_CORES, h=2)
            K4 = KsT[:, :].rearrange("p (a h t) -> p a h t", a=N_CORES, h=2)
            Aq4 = A_q[:, :].rearrange("p (a h t) -> p a h t", a=N_CORES, h=2)
            Ak4 = A_k[:, :].rearrange("p (a h t) -> p a h t", a=N_CORES, h=2)
            nc.vector.tensor_scalar(A_q, A_k, csb[:, 0:1], None, mult)
            nc.vector.tensor_scalar(A_k, A_k, 1.0 / 127.0, None, mult)
            nc.vector.tensor_scalar(Av, Av, csb[:, 1:2], None, mult)

            wq = gath.tile([P, DK, P], f16)
            nc.sync.dma_start(wq, WqT.ap().rearrange("(o p) m -> p o m", p=P))
            wk = gath.tile([P, DK, P], f16)
            nc.sync.dma_start(wk, WkT.ap().rearrange("(o p) m -> p o m", p=P))
            wv = gath.tile([P, DK, P], f16)
            nc.sync.dma_start(wv, WvT.ap().rearrange("(o p) m -> p o m", p=P))

            nc.vector.memset(V_A[:, :, 64:65], 1.0)
            nc.vector.memset(V_B[:, :, 64:65], 1.0)

            for half in range(2):
                for peer in range(N_CORES):
                    stg = workB.tile([P, DK, HL], i8, tag="stg")
                    if half == 0:
                        nc.sync.dma_start(
                            stg, yg0[peer, 0:D, :]
                            .rearrange("(o p) t -> p o t", p=P))
                    else:
                        nc.sync.dma_start(stg, yv1[peer])
                    hk = DK // 2
                    nc.scalar.copy(yTg[:, 0:hk, peer, half, :], stg[:, 0:hk])
                    nc.vector.tensor_copy(yTg[:, hk:DK, peer, half, :],
                                          stg[:, hk:DK])
                for j in range(NSL):
                    pl = slice(j * PPS, (j + 1) * PPS)
                    psq = psQK.tile([P, SLW], f32, tag="psq")
                    for kc in range(DK):
                        nc.tensor.matmul(psq, wq[:, kc],
                                         yTg[:, kc, pl, half, :],
                                         start=(kc == 0), stop=(kc == DK - 1))
                    nc.vector.tensor_tensor(Q4[:, pl, half, :], psq,
                                            Aq4[:, pl, half, :], mult)
                    psk = psQK.tile([P, SLW], f32, tag="psk")
                    for kc in range(DK):
                        nc.tensor.matmul(psk, wk[:, kc],
                                         yTg[:, kc, pl, half, :],
                                         start=(kc == 0), stop=(kc == DK - 1))
                    nc.vector.tensor_tensor(K4[:, pl, half, :], psk,
                                            Ak4[:, pl, half, :], mult)
                if HL >= P:
                    for peer in range(N_CORES):
                        for i in range(HL // P):
                            vt = peer * NT + half * (HL // P) + i
                            psv = psV.tile([P, P], f32, tag="psv")
                            for kc in range(DK):
                                nc.tensor.matmul(
                                    psv,
                                    yTg[:, kc, peer, half, i * P:(i + 1) * P],
                                    wv[:, kc],
                                    start=(kc == 0), stop=(kc == DK - 1))
                            nc.vector.tensor_scalar(
                                V_A[:, vt, 0:64], psv[:, 0:64],
                                Av[:, vt:vt + 1], None, mult)
                            nc.vector.tensor_scalar(
                                V_B[:, vt, 0:64], psv[:, 64:128],
                                Av[:, vt:vt + 1], None, mult)
                elif half == 1:
                    # halves are sub-tile: do V once both halves are in
                    for peer in range(N_CORES):
                        vt = peer
                        psv = psV.tile([P, P], f32, tag="psv")
                        for kc in range(DK):
                            nc.tensor.matmul(
                                psv, yTg[:, kc, peer, :, :], wv[:, kc],
                                start=(kc == 0), stop=(kc == DK - 1))
                        nc.vector.tensor_scalar(
                            V_A[:, vt, 0:64], psv[:, 0:64],
                            Av[:, vt:vt + 1], None, mult)
                        nc.vector.tensor_scalar(
                            V_B[:, vt, 0:64], psv[:, 64:128],
                            Av[:, vt:vt + 1], None, mult)

        # ---------------- Phase C: attention ------------------------------
        sB.__exit__(None, None, None)
        wo = fin.tile([P, DK, D], f16)
        nc.sync.dma_start(wo, WoT.ap().rearrange("(o p) n -> p o n", p=P))

        with tc.tile_pool(name="workC", bufs=3) as workC, \
             tc.tile_pool(name="poSB", bufs=2) as poSB, \
             tc.tile_pool(name="psS", bufs=2, space="PSUM") as psS_pool, \
             tc.tile_pool(name="psO", bufs=2, space="PSUM") as psO_pool:
            sC = nc.named_scope("phaseC"); sC.__enter__()
            a2a_in_v = a2a_in.rearrange("(a h r) l -> a h r l", a=N_CORES, h=2)
            for b in range(B):
                for qb in range(NQB):
                    q0 = b * T + qb * QB
                    qs = QsT[:, q0:q0 + QB]
                    poA = psO_pool.tile([P, QB], f32, tag="poA")
                    poB = psO_pool.tile([P, QB], f32, tag="poB")
                    for kt in range(NKT):
                        k0 = b * T + kt * P
                        kst = KsT[:, k0:k0 + P]
                        ps = psS_pool.tile([P, 1024], f32, tag="S")
                        nc.tensor.matmul(ps[:, 0:512], kst[0:64],
                                         qs[0:64],
                                         start=True, stop=True)
                        nc.tensor.matmul(ps[:, 512:1024], kst[64:128],
                                         qs[64:128],
                                         start=True, stop=True)
                        ex = workC.tile([P, 1024], f16, tag="ex")
                        nc.scalar.activation(ex, ps, Exp)
                        vt = b * NKT + kt
                        nc.tensor.matmul(poA[0:65], V_A[:, vt], ex[:, 0:512],
                                         start=(kt == 0), stop=(kt == NKT - 1))
                        nc.tensor.matmul(poB[0:65], V_B[:, vt], ex[:, 512:1024],
                                         start=(kt == 0), stop=(kt == NKT - 1))
                    # drain unnormalized outputs + denominator rows to the A2A
                    for head, po in ((0, poA), (1, poB)):
                        sbpo = poSB.tile([65, QB], f32, tag=f"sbpo{head}")
                        nc.vector.tensor_copy(sbpo, po[0:65])
                        for j in range(PPQ):
                            peer = (q0 + j * L) // L
                            nc.sync.dma_start(
                                a2a_in_v[peer, head, :, 0:min(L, QB)],
                                sbpo[:, j * L:j * L + min(L, QB)])

            sC.__exit__(None, None, None)
            sA2A = nc.named_scope("a2a"); sA2A.__enter__()
            nc.gpsimd.collective_compute(
                "AllToAll", mybir.AluOpType.bypass, replica_groups=GROUPS,
                ins=[a2a_in.opt()], outs=[a2a_out.opt()])
            sA2A.__exit__(None, None, None)

        attn.release()

        # ---------------- Phase D: normalize + quant + output projection --
        with tc.tile_pool(name="workD", bufs=4) as workD, \
             tc.tile_pool(name="finD", bufs=1) as finD, \
             tc.tile_pool(name="psD", bufs=2, space="PSUM") as psD, \
             tc.tile_pool(name="psTD", bufs=2, space="PSUM") as psTD:
            sD = nc.named_scope("phaseD"); sD.__enter__()
            a2a_out_v = a2a_out.rearrange("(a h r) l -> a h r l", a=N_CORES, h=2)
            # value rows: [2 heads x 64] = 128 partitions per source core
            sv = finD.tile([P, N_CORES, L], f32)
            for s in range(N_CORES):
                nc.sync.dma_start(sv[0:64, s, :], a2a_out_v[s, 0, 0:64, :])
                nc.sync.dma_start(sv[64:128, s, :], a2a_out_v[s, 1, 0:64, :])
            # denominator rows: [16 (s,h), L]
            dsb = finD.tile([2 * N_CORES, L], f32)
            for s in range(N_CORES):
                nc.sync.dma_start(dsb[2 * s:2 * s + 2, :],
                                  a2a_out_v[s, :, 64, :])

            # transpose to token-major
            outf = finD.tile([P, NT, D], f32)
            for s in range(N_CORES):
                for tt in range(NT):
                    pst = psTD.tile([P, P], f32, tag="tr")
                    nc.tensor.transpose(pst, sv[:, s, tt * P:(tt + 1) * P],
                                        ident32)
                    if s % 2 == 0:
                        nc.vector.tensor_copy(outf[:, tt, s * P:(s + 1) * P],
                                              pst)
                    else:
                        nc.scalar.copy(outf[:, tt, s * P:(s + 1) * P], pst)
            dT = finD.tile([P, NT * 2 * N_CORES], f32)
            for tt in range(NT):
                pst = psTD.tile([P, 2 * N_CORES], f32, tag="tr")
                nc.tensor.transpose(pst, dsb[:, tt * P:(tt + 1) * P],
                                    ident32[0:2 * N_CORES, 0:2 * N_CORES])
                nc.vector.tensor_copy(
                    dT[:, tt * 2 * N_CORES:(tt + 1) * 2 * N_CORES], pst)
            rdT = finD.tile([P, NT * 2 * N_CORES], f32)
            nc.vector.reciprocal(rdT, dT)

            y2T = finD.tile([P, DK, L], f16)
            a2r = finD.tile([P, NT], f32)
            for tt in range(NT):
                onrm = workD.tile([P, D], f32, tag="onrm")
                for hb in range(2 * N_CORES):
                    rsc = rdT[:, tt * 2 * N_CORES + hb:tt * 2 * N_CORES + hb + 1]
                    if hb % 2 == 0:
                        nc.vector.tensor_scalar(
                            onrm[:, hb * 64:(hb + 1) * 64],
                            outf[:, tt, hb * 64:(hb + 1) * 64], rsc, None, mult)
                    else:
                        nc.scalar.activation(
                            onrm[:, hb * 64:(hb + 1) * 64],
                            outf[:, tt, hb * 64:(hb + 1) * 64],
                            mybir.ActivationFunctionType.Copy, 0.0, rsc)
                am2 = workD.tile([P, 1], f32, tag="am2")
                nc.vector.reduce_max(am2, onrm, axis=X,
                                     apply_absolute_value=True)
                nc.vector.tensor_scalar_max(am2, am2, 1e-5)
                nc.vector.tensor_tensor(a2r[:, tt:tt + 1], am2, csb[:, 2:3],
                                        mult)
                rec = workD.tile([P, 1], f32, tag="recD")
                nc.vector.reciprocal(rec, am2)
                s127 = workD.tile([P, 1], f32, tag="s127D")
                nc.vector.tensor_scalar_mul(s127, rec, 127.0)
                tmp = workD.tile([P, D], f32, tag="tmpD")
                nc.scalar.activation(tmp, onrm,
                                     mybir.ActivationFunctionType.Copy,
                                     MAGIC, s127)
                y2 = workD.tile([P, D], f16, tag="y2")
                nc.vector.tensor_scalar(y2, tmp, MAGIC, None, subtract)
                for kc in range(DK):
                    pst = psTD.tile([P, P], f16, tag="tr")
                    nc.tensor.transpose(pst, y2[:, kc * P:(kc + 1) * P],
                                        ident16)
                    if kc % 2 == 0:
                        nc.vector.tensor_copy(
                            y2T[:, kc, tt * P:(tt + 1) * P], pst)
                    else:
                        nc.scalar.copy(y2T[:, kc, tt * P:(tt + 1) * P], pst)
            for tt in range(NT):
                psz = psD.tile([P, D], f32, tag="psz")
                for nh in range(2):
                    for kc in range(DK):
                        nc.tensor.matmul(psz[:, nh * 512:(nh + 1) * 512],
                                         y2T[:, kc, tt * P:(tt + 1) * P],
                                         wo[:, kc, nh * 512:(nh + 1) * 512],
                                         start=(kc == 0), stop=(kc == DK - 1))
                zsb = workD.tile([P, D], f32, tag="zsb")
                nc.scalar.activation(zsb, psz,
                                     mybir.ActivationFunctionType.Copy,
                                     0.0, a2r[:, tt:tt + 1])
                nc.sync.dma_start(z.ap()[tt * P:(tt + 1) * P, :], zsb)

        sD.__exit__(None, None, None)
        fin.release()
        dram.release()
        cpool.release()

    nc.compile()
    return nc


def _get_nc(T):
    if T not in _BUILD_CACHE:
        _BUILD_CACHE[T] = _build(T)
    return _BUILD_CACHE[T]


def _wquant(w):
    # reference: scale = 1/clip(mean|w|,1e-5); u = clip(round(w*scale),-1,1)/scale
    scale = np.float32(1.0) / np.maximum(
        np.float32(np.mean(np.abs(w), dtype=np.float64)), np.float32(1e-5))
    u = np.clip(np.rint(w * scale), -1, 1).astype(np.float32)
    return u, np.float32(1.0) / scale  # ternary, dequant scale (= clipped mean)


def kernel(x, mask, Wq, Wk, Wv, Wo, H):
    from concourse.bass_utils import run_bass_kernel_spmd

    x = np.asarray(x, np.float32)
    Wq = np.asarray(Wq, np.float32); Wk = np.asarray(Wk, np.float32)
    Wv = np.asarray(Wv, np.float32); Wo = np.asarray(Wo, np.float32)
    H = np.asarray(H, np.float32)
    Bx, T, Dx = x.shape
    BT = Bx * T
    L = BT // N_CORES

    nc = _get_nc(T)

    xf = x.reshape(BT, Dx)
    x_hi = xf.astype(np.float16)
    x_lo = (xf - x_hi.astype(np.float32)).astype(np.float16)
    H16 = H.astype(np.float16)

    uq, cq = _wquant(Wq); uk, ck = _wquant(Wk)
    uv, cv = _wquant(Wv); uo, co = _wquant(Wo)
    uqT = np.ascontiguousarray(uq.T.astype(np.float16))
    ukT = np.ascontiguousarray(uk.T.astype(np.float16))
    uvT = np.ascontiguousarray(uv.T.astype(np.float16))
    uoT = np.ascontiguousarray(uo.T.astype(np.float16))

    c0 = np.float32(cq) * np.float32(ck) / (np.float32(np.sqrt(DH)) * np.float32(127.0))
    c1 = np.float32(cv) / np.float32(127.0)
    c2 = np.float32(co) / np.float32(127.0)
    consts = np.array([[c0, c1, c2, 0.0]], np.float32)

    in_maps = []
    for c in range(N_CORES):
        rows = slice(c * L, (c + 1) * L)
        cols = slice(c * P, (c + 1) * P)
        in_maps.append({
            "xT_hi": np.ascontiguousarray(x_hi[rows].T),
            "xT_lo": np.ascontiguousarray(x_lo[rows].T),
            "Hm": H16,
            "WqT": np.ascontiguousarray(uqT[:, cols]),
            "WkT": np.ascontiguousarray(ukT[:, cols]),
            "WvT": np.ascontiguousarray(uvT[:, cols]),
            "WoT": uoT,
            "consts": consts,
        })

    res = run_bass_kernel_spmd(nc, in_maps, core_ids=list(range(N_CORES)))
    kernel.last_results = res
    z = np.concatenate([res.results[c]["z"] for c in range(N_CORES)], axis=0)
    return z.reshape(Bx, T, Dx).astype(np.float32)

